# revision 22
# baseline (speedup 1.0000x reference)
"""MultiHeadAttention + residual + LayerNorm Trainium2 kernel (8 NeuronCores).

Sharding: core c handles batch b = c//2 and query half h = c%2 (1024 queries).
No cross-core communication.

All heavy matmuls run in fp8(e4m3) DoubleRow perf mode (two 128-deep k-tiles
per instruction at 0.5 cycles/row = 4x bf16 throughput per accumulation pair):
  - K^T / Q^T projections with HOST-PERMUTED weight columns so each head's
    64-deep score contraction lands as two 32-row k-tiles on partition group
    (h%4)*32..(h%4+1)*32, free dims (h_hi, dh_hi, token).
  - scores S^T[k, q] = K Q^T as one DoubleRow matmul per (head, key-chunk).
  - softmax exp is split across ScalarE (native Exp -> fp8 out) and DVE
    (Schraudolph: bits = s*A+B as int8, bitcast e4m3; the constant bias only
    multiplies all p's by a constant which cancels in the softmax divide).
  - ctx~^T = [V|1]^T P^T DoubleRow over key-chunk pairs; row 64 = denominator.
  - denominators: DMA-spread to 16 partitions, fast-reciprocal, DMA back,
    PE rank-1 broadcast, fused scale+fp8-cast on DVE -> ctxt.
  - out-projection produces y[q, dout] (queries on partitions) so LayerNorm
    runs over the FREE dim: residual-add with accum_out gives the mean for
    free, ScalarE Square/Pool give sum-of-squares, finalize is
    ScalarE-Identity(scale=rstd, bias=-mu*rstd) + gamma/beta broadcast mults.
  - b_o + x is folded host-side into the residual input; b_q/b_k fold into
    the PSUM->SBUF copies (per-partition bias); b_v is added into V so the
    softmax-weighted average carries it exactly.
"""

import os
from contextlib import ExitStack

import numpy as np

import concourse.bass as bass
import concourse.mybir as mybir
import concourse.tile as tile

B, S, D, H, DH = 4, 2048, 512, 8, 64
SQ = S // 2          # local queries per core
NCORES = 8
P = 128
NC_D = D // P        # 4 chunks of the feature dim
NC_S = S // P        # 16 key chunks
SCALE = float(1.0 / np.sqrt(np.float32(D)))
EPS = 1e-5
LN2 = float(np.log(2.0))
A8 = 8.0 / LN2                     # Schraudolph slope for e4m3 bit space
B8 = 7.0 * 8.0 - 0.34 + 0.5        # bias: exp-field offset, ripple center,
                                   # +0.5 for truncating converts (any
                                   # constant shift cancels in softmax)

F32 = mybir.dt.float32
F32R = mybir.dt.float32r
BF16 = mybir.dt.bfloat16
FP8 = mybir.dt.float8e4
I8 = mybir.dt.int8
ALU = mybir.AluOpType
AFT = mybir.ActivationFunctionType
DR = mybir.MatmulPerfMode.DoubleRow

# exp engine rotation per key-chunk: ScalarE native exp is ~1.3x faster per
# chunk than the DVE Schraudolph, so it takes 10 of 16
EXP_SC = (0, 2, 3, 5, 6, 8, 9, 11, 12, 14)
# remaining 6 on DVE: (1, 4, 7, 10, 13, 15) - the final two kc land on
# different engines so the last pt completes without serializing


def _split_multiwait_json(bir, cap=1):
    """The walrus build here encodes at most one sync-wait command per
    instruction. Hoist excess waits onto preceding single-wait NoOps on the
    same engine - engine streams execute in order, so waiting earlier is
    always safe."""
    n = 0
    for fn in bir.get("functions", []):
        for bb in fn.get("blocks", []):
            out = []
            for ins in bb.get("instructions", []):
                si = ins.get("sync_info")
                waits = (si or {}).get("on_wait") or []
                if len(waits) > cap:
                    extra, si["on_wait"] = waits[:-cap], waits[-cap:]
                    for i in range(0, len(extra), cap):
                        n += 1
                        out.append(
                            {
                                "debug": ins.get("debug", 0),
                                "engine": ins["engine"],
                                "ins": [],
                                "outs": [],
                                "name": f"{ins['name']}-wsplit{n}",
                                "opcode": "NoOp",
                                "sync_info": {
                                    "on_wait": extra[i : i + cap],
                                    "on_update": [],
                                },
                            }
                        )
                out.append(ins)
            bb["instructions"] = out
    return bir


def _patch_serialization(nc):
    import orjson

    orig = nc.to_json_bytes

    def to_json_bytes_split():
        return orjson.dumps(_split_multiwait_json(orjson.loads(orig())))

    nc.to_json_bytes = to_json_bytes_split
    return nc


def build_nc():
    nc = bass.Bass("TRN2", target_bir_lowering=False)

    xt_d = nc.dram_tensor("xt", [D, S], FP8, kind="ExternalInput")
    xq_d = nc.dram_tensor("xq", [D, SQ], FP8, kind="ExternalInput")
    xqbo_d = nc.dram_tensor("xqbo", [SQ, D], F32, kind="ExternalInput")
    wk_d = nc.dram_tensor("wkt", [D, D], FP8, kind="ExternalInput")
    wq_d = nc.dram_tensor("wqt", [D, D], FP8, kind="ExternalInput")
    wv_d = nc.dram_tensor("wvt", [D, D], FP8, kind="ExternalInput")
    wo_d = nc.dram_tensor("wot", [D, D], FP8, kind="ExternalInput")
    bkc_d = nc.dram_tensor("bkc", [NC_D, P], F32, kind="ExternalInput")
    bqc_d = nc.dram_tensor("bqc", [NC_D, P], F32, kind="ExternalInput")
    sel2_d = nc.dram_tensor("sel2", [2, P], F32R, kind="ExternalInput")
    gamma_d = nc.dram_tensor("gamma", [D], F32R, kind="ExternalInput")
    beta_d = nc.dram_tensor("beta", [D], F32R, kind="ExternalInput")
    ytd = nc.dram_tensor("ytd", [SQ, D], F32, kind="ExternalOutput")

    with (
        tile.TileContext(nc) as tc,
        ExitStack() as ctx,
        nc.allow_low_precision(reason="fp8 matmuls feed a diffuse softmax"),
    ):
        singles = ctx.enter_context(tc.tile_pool(name="singles", bufs=1))
        ptpool = ctx.enter_context(tc.tile_pool(name="ptpool", bufs=3))
        dpool = ctx.enter_context(tc.tile_pool(name="dpool", bufs=2))
        tpool = ctx.enter_context(tc.tile_pool(name="tpool", bufs=2))
        ps_sc = ctx.enter_context(tc.tile_pool(name="ps_sc", bufs=2, space="PSUM"))
        ps_ct = ctx.enter_context(tc.tile_pool(name="ps_ct", bufs=3, space="PSUM"))
        ps_y = ctx.enter_context(tc.tile_pool(name="ps_y", bufs=1, space="PSUM"))

        # ---- input DMAs (K-proj path first) ----
        xt = singles.tile([P, NC_D, S], FP8)
        wk = singles.tile([P, NC_D, D], FP8)
        nc.sync.dma_start(wk[:], wk_d[:, :].rearrange("(c p) f -> p c f", p=P))
        for i in range(4):
            ts_ = slice(i * 512, (i + 1) * 512)
            nc.sync.dma_start(
                xt[:, :, ts_],
                xt_d[:, :].rearrange("(c p) t -> p c t", p=P)[:, :, ts_],
            )
        bkc = singles.tile([P, NC_D], F32)
        bqc = singles.tile([P, NC_D], F32)
        nc.sync.dma_start(bkc[:], bkc_d[:, :].rearrange("m p -> p m"))
        nc.sync.dma_start(bqc[:], bqc_d[:, :].rearrange("m p -> p m"))
        wv = singles.tile([P, NC_D, D], FP8)
        nc.scalar.dma_start(wv[:], wv_d[:, :].rearrange("(c p) f -> p c f", p=P))
        wq = singles.tile([P, NC_D, D], FP8)
        nc.scalar.dma_start(wq[:], wq_d[:, :].rearrange("(c p) f -> p c f", p=P))
        xq = singles.tile([P, NC_D, SQ], FP8)
        nc.scalar.dma_start(xq[:], xq_d[:, :].rearrange("(c p) t -> p c t", p=P))
        wot = singles.tile([P, NC_D, D], FP8)
        nc.scalar.dma_start(wot[:], wo_d[:, :].rearrange("(c p) f -> p c f", p=P))
        xqbo = singles.tile([P, SQ // P, D], F32)
        nc.scalar.dma_start(
            xqbo[:], xqbo_d[:, :].rearrange("(qc p) d -> p qc d", p=P)
        )
        gamma_row = singles.tile([1, D], F32R)
        beta_row = singles.tile([1, D], F32R)
        nc.sync.dma_start(gamma_row[:], gamma_d[:][None, :])
        nc.sync.dma_start(beta_row[:], beta_d[:][None, :])

        # ---- persistent SBUF ----
        kt4 = singles.tile([P, NC_D, S], FP8)   # [dk%128, dk//128, t]
        qt3 = singles.tile([P, H, SQ], FP8)     # head h real rows at (h%2)*64
        vext = singles.tile([P, NC_S, H, P], FP8)
        ctxt = singles.tile([P, NC_D, SQ], FP8)    # [dv%128, h//2, q]
        y_sb = singles.tile([P, SQ // P, D], F32)  # residual+proj, LN input
        msum = singles.tile([P, SQ // P], F32)
        msqs = singles.tile([P, SQ // P], F32)

        # consts
        ones_col_r = singles.tile([1, P], F32R)
        eps_col = singles.tile([P, 1], F32)
        f32tmp = singles.tile([P, 512], F32)
        nc.vector.memset(f32tmp[:], 1.0)
        nc.vector.tensor_copy(ones_col_r[:], f32tmp[0:1, 0:P])
        nc.vector.memset(eps_col[:], EPS)
        nc.vector.memset(vext[:, :, :, DH:P], 1.0)  # ones -> denominators
        nc.vector.memset(qt3[0:DH, 1::2, :], 0.0)  # zero rows select the
        nc.vector.memset(qt3[DH:P, 0::2, :], 0.0)  # head in score matmuls
        sel2 = singles.tile([2, P], F32R)  # rank-2 lhsT: head parity -> rows
        nc.sync.dma_start(sel2[:], sel2_d[:, :])

        # broadcast rows -> [128, 512] via rank-1 matmuls
        gamma_b = singles.tile([P, D], F32)
        beta_b = singles.tile([P, D], F32)
        for row, dst in ((gamma_row, gamma_b), (beta_row, beta_b)):
            ps = ps_y.tile([P, D], F32, tag="y")
            nc.tensor.matmul(ps[:], ones_col_r[:], row[:], start=True, stop=True)
            nc.scalar.activation(dst[:], ps[:], AFT.Copy)

        # ---- K projection -> kt2 (DoubleRow, bias folded into copy) ----
        # m-chunk m covers dk rows {(m//2*4 + p//32)*64 + (m%2)*32 + p%32}
        for nb in range(4):
            for m in range(NC_D):
                ps = ps_sc.tile([P, 512], F32, tag="sc")
                for cp in range(2):
                    nc.tensor.matmul(
                        ps[:],
                        wk[:, 2 * cp : 2 * cp + 2, m * P : (m + 1) * P],
                        xt[:, 2 * cp : 2 * cp + 2, nb * 512 : (nb + 1) * 512],
                        start=(cp == 0),
                        stop=(cp == 1),
                        perf_mode=DR,
                    )
                nc.scalar.activation(
                    kt4[:, m, nb * 512 : (nb + 1) * 512],
                    ps[:],
                    AFT.Identity,
                    bias=bkc[:, m : m + 1],
                )

        # ---- V projection -> vext (DoubleRow, bias via broadcast add) ----
        for t in range(NC_S):
            ps = ps_sc.tile([P, 512], F32, tag="sc")
            for cp in range(2):
                nc.tensor.matmul(
                    ps[:],
                    xt[:, 2 * cp : 2 * cp + 2, t * P : (t + 1) * P],
                    wv[:, 2 * cp : 2 * cp + 2, :],
                    start=(cp == 0),
                    stop=(cp == 1),
                    perf_mode=DR,
                )
            nc.vector.tensor_copy(
                vext[:, t, :, 0:DH],
                ps[:].rearrange("p (h d) -> p h d", h=H),
            )

        # ---- Q projection -> qt2 ----
        for m in range(NC_D):
            for nb in range(2):
                ps = ps_sc.tile([P, 512], F32, tag="sc")
                for cp in range(2):
                    nc.tensor.matmul(
                        ps[:],
                        wq[:, 2 * cp : 2 * cp + 2, m * P : (m + 1) * P],
                        xq[:, 2 * cp : 2 * cp + 2, nb * 512 : (nb + 1) * 512],
                        start=(cp == 0),
                        stop=(cp == 1),
                        perf_mode=DR,
                    )
                qslice = slice(nb * 512, (nb + 1) * 512)
                nc.scalar.activation(
                    qt3[0:DH, 2 * m, qslice],
                    ps[0:DH, :],
                    AFT.Identity,
                    bias=bqc[0:DH, m : m + 1],
                )
                nc.vector.tensor_scalar(
                    qt3[DH:P, 2 * m + 1, qslice],
                    ps[DH:P, :],
                    bqc[DH:P, m : m + 1],
                    None,
                    ALU.add,
                )

        # ---- attention ----
        def attention_pair(qb, pair, hooks=()):
            """scores + exp + ctx accumulation for heads (2*pair, 2*pair+1);
            returns the two ctx psum tiles (row 64 = softmax denominator).
            hooks: dict kc -> callback, emitted mid-loop so cross-engine
            work lands in each engine's stream after its deps are met."""
            qs = slice(qb * 512, (qb + 1) * 512)
            cts = [
                ps_ct.tile([P, 512], F32, tag="ct", name=f"ct{qb}{pair}{i}")
                for i in range(2)
            ]
            pt = None
            for kc in range(NC_S):
                if kc in hooks:
                    hooks[kc]()
                j, jj = divmod(kc, 2)
                if jj == 0:
                    pt = ptpool.tile([P, 2, 2, 512], FP8, tag="pt")
                sc = ps_sc.tile([P, 2, 512], F32, tag="sc")
                for hh in range(2):
                    h = 2 * pair + hh
                    nc.tensor.matmul(
                        sc[:, hh, :],
                        kt4[:, pair, kc * P : (kc + 1) * P],
                        qt3[:, h, qs],
                        start=True,
                        stop=True,
                    )
                if kc in EXP_SC:
                    nc.scalar.activation(
                        pt[:, jj, :, :], sc[:], AFT.Exp, scale=SCALE
                    )
                else:
                    nc.vector.tensor_scalar(
                        pt[:, jj, :, :].bitcast(I8),
                        sc[:],
                        SCALE * A8,
                        B8,
                        ALU.mult,
                        ALU.add,
                    )
                if jj == 1 and j < NC_S // 2 - 1:
                    for hh in range(2):
                        nc.tensor.matmul(
                            cts[hh][:, :],
                            vext[:, kc - 1 : kc + 1, 2 * pair + hh, :],
                            pt[:, :, hh, :],
                            start=(j == 0),
                            stop=False,
                            perf_mode=DR,
                        )
            pt_last = pt

            def ctx_last():
                for hh in range(2):
                    nc.tensor.matmul(
                        cts[hh][:, :],
                        vext[:, NC_S - 2 : NC_S, 2 * pair + hh, :],
                        pt_last[:, :, hh, :],
                        start=False,
                        stop=True,
                        perf_mode=DR,
                    )
            return cts, ctx_last

        def evac_pair(qb, pair, cts):
            """Free the ctx psums fast: raw fp8 ctx into ctxt (scaled in
            place later), exact f32 denominator rows, reciprocal via the
            16-partition DMA spread. Returns recrow for the deferred scale."""
            qs = slice(qb * 512, (qb + 1) * 512)
            nc.scalar.activation(ctxt[0:DH, pair, qs], cts[0][0:DH, :], AFT.Copy)
            nc.vector.tensor_copy(ctxt[DH:P, pair, qs], cts[1][0:DH, :])
            denrow = dpool.tile([1, 2, 512], F32, tag="denrow")
            nc.scalar.activation(denrow[:, 0, :], cts[0][DH : DH + 1, :], AFT.Copy)
            nc.vector.tensor_copy(denrow[:, 1, :], cts[1][DH : DH + 1, :])
            dsq = dpool.tile([16, 64], F32, tag="dsq")
            nc.sync.dma_start(dsq[:], denrow[:])
            nc.vector.reciprocal(dsq[:], dsq[:])
            recrow = dpool.tile([2, 512], F32, tag="recrow")
            nc.sync.dma_start(recrow[:], dsq[:])
            return recrow

        def finish_pair(qb, pair, recrow):
            """Deferred ~1 pair: broadcast 1/den and scale ctxt in place."""
            qs = slice(qb * 512, (qb + 1) * 512)
            rb = ps_y.tile([P, 512], F32, tag="y", name=f"rb{qb}{pair}")
            nc.tensor.matmul(
                rb[:],
                sel2[:],
                recrow[:].bitcast(F32R),
                start=True,
                stop=True,
            )
            rb_sb = tpool.tile([P, 512], F32, tag="rb")
            nc.scalar.activation(rb_sb[:], rb[:], AFT.Copy)
            cslice = ctxt[:, pair, qs]
            nc.vector.tensor_tensor(cslice, cslice, rb_sb[:], ALU.mult)

        def outproj_chunk(qc):
            """y[qc*128:(qc+1)*128, :] = ctx @ wo^T + (x + bo); fused stats."""
            ps = ps_y.tile([P, D], F32, tag="y", name=f"y{qc}")
            for cp in range(2):
                nc.tensor.matmul(
                    ps[:],
                    ctxt[:, 2 * cp : 2 * cp + 2, qc * P : (qc + 1) * P],
                    wot[:, 2 * cp : 2 * cp + 2, :],
                    start=(cp == 0),
                    stop=(cp == 1),
                    perf_mode=DR,
                )
            nc.vector.scalar_tensor_tensor(
                y_sb[:, qc, :],
                ps[:],
                0.0,
                xqbo[:, qc, :],
                ALU.bypass,
                ALU.add,
                accum_out=msum[:, qc : qc + 1],
            )
            sq = tpool.tile([P, D], BF16, tag="sq")
            nc.scalar.activation(
                sq[:],
                y_sb[:, qc, :],
                AFT.Square,
                accum_out=msqs[:, qc : qc + 1],
            )

        def ln_stats(qb):
            """rstd/bias for the 4 chunks of query block qb: [128, 4] ops."""
            cs = slice(qb * 4, (qb + 1) * 4)
            mu = dpool.tile([P, 4], F32, tag="mu", name=f"mu{qb}")
            var = dpool.tile([P, 4], F32, tag="var", name=f"var{qb}")
            rstd = singles.tile([P, 4], F32, tag=f"rstd{qb}")
            nmr = singles.tile([P, 4], F32, tag=f"nmr{qb}")
            nc.vector.tensor_scalar_mul(mu[:], msum[:, cs], 1.0 / D)
            nc.vector.tensor_tensor(var[:], mu[:], mu[:], ALU.mult)
            nc.vector.scalar_tensor_tensor(
                var[:], msqs[:, cs], 1.0 / D, var[:], ALU.mult, ALU.subtract
            )
            nc.scalar.activation(rstd[:], var[:], AFT.Sqrt, bias=eps_col[:, 0:1])
            nc.vector.reciprocal(rstd[:], rstd[:])
            nc.vector.scalar_tensor_tensor(
                nmr[:], mu[:], -1.0, rstd[:], ALU.mult, ALU.mult
            )
            return rstd, nmr

        def ln_finalize_chunk(qb, qc, rstd, nmr):
            i = qc - qb * 4
            t1 = tpool.tile([P, D], F32, tag="t1")
            nc.scalar.activation(
                t1[:],
                y_sb[:, qc, :],
                AFT.Identity,
                bias=nmr[:, i : i + 1],
                scale=rstd[:, i : i + 1],
            )
            t2 = tpool.tile([P, D], F32, tag="t2")
            yo = tpool.tile([P, D], F32, tag="yo")
            if qb == 0 or qc % 2 == 0:
                nc.vector.tensor_tensor(t2[:], t1[:], gamma_b[:], ALU.mult)
                eng = nc.gpsimd if qb == 0 else nc.vector
                eng.tensor_tensor(yo[:], t2[:], beta_b[:], ALU.add)
            else:
                nc.gpsimd.tensor_tensor(t2[:], t1[:], gamma_b[:], ALU.mult)
                nc.gpsimd.tensor_tensor(yo[:], t2[:], beta_b[:], ALU.add)
            nc.sync.dma_start(ytd[qc * P : (qc + 1) * P, :], yo[:])

        # attention: pair p's evacuation (kc=4 hook) and normalize/tail work
        # (kc=11 hook) are emitted inside pair p+1/p+2's kc loop, so no
        # engine's in-order stream blocks on a dependency that isn't ready
        cts_d = {}
        rr_d = {}
        ln0 = {}

        def ctxlast_hook(qb, pair):
            def fn():
                cts_d[(qb, pair)][1]()
            return fn

        def evac_hook(qb, pair):
            def fn():
                rr_d[(qb, pair)] = evac_pair(qb, pair, cts_d.pop((qb, pair))[0])
            return fn

        def finish_hook(qb, pair, extra=None):
            def fn():
                finish_pair(qb, pair, rr_d.pop((qb, pair)))
                if extra is not None:
                    extra()
            return fn

        def outproj0_all():
            for qc in range(4):
                outproj_chunk(qc)

        def ln0_block():
            ln0["r"] = ln_stats(0)
            for qc in range(4):
                ln_finalize_chunk(0, qc, *ln0["r"])

        schedule = [
            (0, 0, {}),
            (0, 1, {2: ctxlast_hook(0, 0), 4: evac_hook(0, 0)}),
            (0, 2, {2: ctxlast_hook(0, 1), 4: evac_hook(0, 1),
                    11: finish_hook(0, 0)}),
            (0, 3, {2: ctxlast_hook(0, 2), 4: evac_hook(0, 2),
                    11: finish_hook(0, 1)}),
            (1, 0, {2: ctxlast_hook(0, 3), 4: evac_hook(0, 3),
                    11: finish_hook(0, 2)}),
            (1, 1, {2: ctxlast_hook(1, 0), 4: evac_hook(1, 0),
                    11: finish_hook(0, 3, outproj0_all)}),
            (1, 2, {2: ctxlast_hook(1, 1), 4: evac_hook(1, 1),
                    11: finish_hook(1, 0)}),
            (1, 3, {2: ctxlast_hook(1, 2), 4: evac_hook(1, 2),
                    11: finish_hook(1, 1), 14: ln0_block}),
        ]
        for qb, pair, hooks in schedule:
            cts_d[(qb, pair)] = attention_pair(qb, pair, hooks)
        finish_pair(1, 2, rr_d.pop((1, 2)))
        cts_d[(1, 3)][1]()
        rr_d[(1, 3)] = evac_pair(1, 3, cts_d.pop((1, 3))[0])
        finish_pair(1, 3, rr_d.pop((1, 3)))
        for qc in range(4, 8):
            outproj_chunk(qc)
        r1 = ln_stats(1)
        for qc in range(4, 8):
            ln_finalize_chunk(1, qc, *r1)

    return _patch_serialization(nc)


_nc_cache = None


def _get_nc():
    global _nc_cache
    if _nc_cache is None:
        _nc_cache = build_nc()
    return _nc_cache


def make_in_maps(x, w_q, b_q, w_k, b_k, w_v, b_v, w_o, b_o, ln_gamma, ln_beta):
    import ml_dtypes

    fp8 = ml_dtypes.float8_e4m3
    f8 = lambda a: np.ascontiguousarray(np.asarray(a, np.float32)).astype(fp8)
    f = lambda a: np.ascontiguousarray(np.asarray(a), dtype=np.float32)

    w_q, w_k, w_v, w_o = (np.asarray(w, np.float32) for w in (w_q, w_k, w_v, w_o))
    b_q, b_k = np.asarray(b_q, np.float32), np.asarray(b_k, np.float32)
    shared = dict(
        wkt=f8(w_k.T),
        wqt=f8(w_q.T),
        wvt=f8(w_v.T),
        wot=f8(w_o.T),
        bkc=f(b_k.reshape(NC_D, P)),
        bqc=f(b_q.reshape(NC_D, P)),
        gamma=f(ln_gamma),
        beta=f(ln_beta),
        sel2=np.kron(np.eye(2, dtype=np.float32), np.ones((1, DH), np.float32)),
    )
    x = np.asarray(x, np.float32)
    # b_o + softmax-weight-sum * b_v @ w_o.T both fold into the residual input
    resid_bias = np.asarray(b_o, np.float32) + np.asarray(b_v, np.float32) @ w_o.T
    in_maps = []
    for c in range(NCORES):
        b, half = divmod(c, 2)
        off = half * SQ
        xb = x[b]
        in_maps.append(
            dict(
                xt=f8(xb.T),
                xq=f8(xb[off : off + SQ].T),
                xqbo=np.ascontiguousarray(xb[off : off + SQ] + resid_bias),
                **shared,
            )
        )
    return in_maps


def assemble(results):
    y = np.empty((B, S, D), np.float32)
    for c in range(NCORES):
        b, half = divmod(c, 2)
        off = half * SQ
        y[b, off : off + SQ, :] = results[c]["ytd"]
    return y


def run(inputs, trace=False, **kwargs):
    from concourse.bass_utils import run_bass_kernel_spmd

    nc = _get_nc()
    in_maps = make_in_maps(**inputs)
    res = run_bass_kernel_spmd(
        nc, in_maps, core_ids=list(range(NCORES)), trace=trace, **kwargs
    )
    return assemble(res.results), res


def kernel(**inputs):
    y, _ = run(inputs, trace=False)
    return y


# revision 23
# speedup vs baseline: 1.2291x; 1.2291x over previous
"""MultiHeadAttention + residual + LayerNorm Trainium2 kernel (8 NeuronCores).

Sharding: core c handles batch b = c//2 and query half h = c%2 (1024 queries).
No cross-core communication.

All heavy matmuls run in fp8(e4m3) DoubleRow perf mode (two 128-deep k-tiles
per instruction at 0.5 cycles/row = 4x bf16 throughput per accumulation pair):
  - K^T / Q^T projections with HOST-PERMUTED weight columns so each head's
    64-deep score contraction lands as two 32-row k-tiles on partition group
    (h%4)*32..(h%4+1)*32, free dims (h_hi, dh_hi, token).
  - scores S^T[k, q] = K Q^T as one DoubleRow matmul per (head, key-chunk).
  - softmax exp is split across ScalarE (native Exp -> fp8 out) and DVE
    (Schraudolph: bits = s*A+B as int8, bitcast e4m3; the constant bias only
    multiplies all p's by a constant which cancels in the softmax divide).
  - ctx~^T = [V|1]^T P^T DoubleRow over key-chunk pairs; row 64 = denominator.
  - denominators: DMA-spread to 16 partitions, fast-reciprocal, DMA back,
    PE rank-1 broadcast, fused scale+fp8-cast on DVE -> ctxt.
  - out-projection produces y[q, dout] (queries on partitions) so LayerNorm
    runs over the FREE dim: residual-add with accum_out gives the mean for
    free, ScalarE Square/Pool give sum-of-squares, finalize is
    ScalarE-Identity(scale=rstd, bias=-mu*rstd) + gamma/beta broadcast mults.
  - b_o + x is folded host-side into the residual input; b_q/b_k fold into
    the PSUM->SBUF copies (per-partition bias); b_v is added into V so the
    softmax-weighted average carries it exactly.
"""

import os
from contextlib import ExitStack

import numpy as np

import concourse.bass as bass
import concourse.mybir as mybir
import concourse.tile as tile

B, S, D, H, DH = 4, 2048, 512, 8, 64
SQ = S // 2          # local queries per core
NCORES = 8
P = 128
NC_D = D // P        # 4 chunks of the feature dim
NC_S = S // P        # 16 key chunks
SCALE = float(1.0 / np.sqrt(np.float32(D)))
EPS = 1e-5
LN2 = float(np.log(2.0))
A8 = 8.0 / LN2                     # Schraudolph slope for e4m3 bit space
B8 = 7.0 * 8.0 - 0.34 + 0.5        # bias: exp-field offset, ripple center,
                                   # +0.5 for truncating converts (any
                                   # constant shift cancels in softmax)

F32 = mybir.dt.float32
F32R = mybir.dt.float32r
BF16 = mybir.dt.bfloat16
FP8 = mybir.dt.float8e4
I8 = mybir.dt.int8
ALU = mybir.AluOpType
AFT = mybir.ActivationFunctionType
DR = mybir.MatmulPerfMode.DoubleRow

# exp engine rotation per key-chunk: ScalarE native exp is ~1.3x faster per
# chunk than the DVE Schraudolph, so it takes 10 of 16
EXP_SC = (0, 2, 3, 5, 6, 8, 9, 11, 12, 14)
# remaining 6 on DVE: (1, 4, 7, 10, 13, 15) - the final two kc land on
# different engines so the last pt completes without serializing


def _split_multiwait_json(bir, cap=1):
    """The walrus build here encodes at most one sync-wait command per
    instruction. Hoist excess waits onto preceding single-wait NoOps on the
    same engine - engine streams execute in order, so waiting earlier is
    always safe."""
    n = 0
    for fn in bir.get("functions", []):
        for bb in fn.get("blocks", []):
            out = []
            for ins in bb.get("instructions", []):
                si = ins.get("sync_info")
                waits = (si or {}).get("on_wait") or []
                if len(waits) > cap:
                    extra, si["on_wait"] = waits[:-cap], waits[-cap:]
                    for i in range(0, len(extra), cap):
                        n += 1
                        out.append(
                            {
                                "debug": ins.get("debug", 0),
                                "engine": ins["engine"],
                                "ins": [],
                                "outs": [],
                                "name": f"{ins['name']}-wsplit{n}",
                                "opcode": "NoOp",
                                "sync_info": {
                                    "on_wait": extra[i : i + cap],
                                    "on_update": [],
                                },
                            }
                        )
                out.append(ins)
            bb["instructions"] = out
    return bir


def _patch_serialization(nc):
    import orjson

    orig = nc.to_json_bytes

    def to_json_bytes_split():
        return orjson.dumps(_split_multiwait_json(orjson.loads(orig())))

    nc.to_json_bytes = to_json_bytes_split
    return nc


def build_nc():
    nc = bass.Bass("TRN2", target_bir_lowering=False)

    xt_d = nc.dram_tensor("xt", [D, S], FP8, kind="ExternalInput")
    xq_d = nc.dram_tensor("xq", [D, SQ], FP8, kind="ExternalInput")
    xqbo_d = nc.dram_tensor("xqbo", [SQ, D], F32, kind="ExternalInput")
    wk_d = nc.dram_tensor("wkt", [D, D], FP8, kind="ExternalInput")
    wq_d = nc.dram_tensor("wqt", [D, D], FP8, kind="ExternalInput")
    wv_d = nc.dram_tensor("wvt", [D, D], FP8, kind="ExternalInput")
    wo_d = nc.dram_tensor("wot", [D, D], FP8, kind="ExternalInput")
    bkc_d = nc.dram_tensor("bkc", [NC_D, P], F32, kind="ExternalInput")
    bqc_d = nc.dram_tensor("bqc", [NC_D, P], F32, kind="ExternalInput")
    sel2_d = nc.dram_tensor("sel2", [2, P], F32R, kind="ExternalInput")
    gamma_d = nc.dram_tensor("gamma", [D], F32R, kind="ExternalInput")
    beta_d = nc.dram_tensor("beta", [D], F32R, kind="ExternalInput")
    ytd = nc.dram_tensor("ytd", [SQ, D], F32, kind="ExternalOutput")

    with (
        tile.TileContext(nc) as tc,
        ExitStack() as ctx,
        nc.allow_low_precision(reason="fp8 matmuls feed a diffuse softmax"),
    ):
        singles = ctx.enter_context(tc.tile_pool(name="singles", bufs=1))
        ptpool = ctx.enter_context(tc.tile_pool(name="ptpool", bufs=3))
        dpool = ctx.enter_context(tc.tile_pool(name="dpool", bufs=2))
        tpool = ctx.enter_context(tc.tile_pool(name="tpool", bufs=2))
        ps_sc = ctx.enter_context(tc.tile_pool(name="ps_sc", bufs=2, space="PSUM"))
        ps_ct = ctx.enter_context(tc.tile_pool(name="ps_ct", bufs=3, space="PSUM"))
        ps_y = ctx.enter_context(tc.tile_pool(name="ps_y", bufs=1, space="PSUM"))

        # ---- input DMAs (K-proj path first) ----
        xt = singles.tile([P, NC_D, S], FP8)
        wk = singles.tile([P, NC_D, D], FP8)
        nc.sync.dma_start(wk[:], wk_d[:, :].rearrange("(c p) f -> p c f", p=P))
        for i in range(4):
            ts_ = slice(i * 512, (i + 1) * 512)
            nc.sync.dma_start(
                xt[:, :, ts_],
                xt_d[:, :].rearrange("(c p) t -> p c t", p=P)[:, :, ts_],
            )
        bkc = singles.tile([P, NC_D], F32)
        bqc = singles.tile([P, NC_D], F32)
        nc.sync.dma_start(bkc[:], bkc_d[:, :].rearrange("m p -> p m"))
        nc.sync.dma_start(bqc[:], bqc_d[:, :].rearrange("m p -> p m"))
        wv = singles.tile([P, NC_D, D], FP8)
        nc.sync.dma_start(wv[:], wv_d[:, :].rearrange("(c p) f -> p c f", p=P))
        wq = singles.tile([P, NC_D, D], FP8)
        nc.sync.dma_start(wq[:], wq_d[:, :].rearrange("(c p) f -> p c f", p=P))
        xq = singles.tile([P, NC_D, SQ], FP8)
        nc.sync.dma_start(xq[:], xq_d[:, :].rearrange("(c p) t -> p c t", p=P))
        wot = singles.tile([P, NC_D, D], FP8)
        nc.sync.dma_start(wot[:], wo_d[:, :].rearrange("(c p) f -> p c f", p=P))
        xqbo = singles.tile([P, SQ // P, D], F32)
        nc.sync.dma_start(
            xqbo[:], xqbo_d[:, :].rearrange("(qc p) d -> p qc d", p=P)
        )
        gamma_row = singles.tile([1, D], F32R)
        beta_row = singles.tile([1, D], F32R)
        nc.sync.dma_start(gamma_row[:], gamma_d[:][None, :])
        nc.sync.dma_start(beta_row[:], beta_d[:][None, :])

        # ---- persistent SBUF ----
        kt4 = singles.tile([P, NC_D, S], FP8)   # [dk%128, dk//128, t]
        qt3 = singles.tile([P, H, SQ], FP8)     # head h real rows at (h%2)*64
        vext = singles.tile([P, NC_S, H, P], FP8)
        ctxt = singles.tile([P, NC_D, SQ], FP8)    # [dv%128, h//2, q]
        y_sb = singles.tile([P, SQ // P, D], F32)  # residual+proj, LN input
        msum = singles.tile([P, SQ // P], F32)
        msqs = singles.tile([P, SQ // P], F32)

        # consts
        ones_col_r = singles.tile([1, P], F32R)
        eps_col = singles.tile([P, 1], F32)
        f32tmp = singles.tile([P, 512], F32)
        nc.vector.memset(f32tmp[:], 1.0)
        nc.vector.tensor_copy(ones_col_r[:], f32tmp[0:1, 0:P])
        nc.vector.memset(eps_col[:], EPS)
        nc.vector.memset(vext[:, :, :, DH:P], 1.0)  # ones -> denominators
        nc.vector.memset(qt3[0:DH, 1::2, :], 0.0)  # zero rows select the
        nc.vector.memset(qt3[DH:P, 0::2, :], 0.0)  # head in score matmuls
        sel2 = singles.tile([2, P], F32R)  # rank-2 lhsT: head parity -> rows
        nc.sync.dma_start(sel2[:], sel2_d[:, :])

        # broadcast rows -> [128, 512] via rank-1 matmuls
        gamma_b = singles.tile([P, D], F32)
        beta_b = singles.tile([P, D], F32)
        for row, dst in ((gamma_row, gamma_b), (beta_row, beta_b)):
            ps = ps_y.tile([P, D], F32, tag="y")
            nc.tensor.matmul(ps[:], ones_col_r[:], row[:], start=True, stop=True)
            nc.scalar.activation(dst[:], ps[:], AFT.Copy)

        # ---- K projection -> kt2 (DoubleRow, bias folded into copy) ----
        # m-chunk m covers dk rows {(m//2*4 + p//32)*64 + (m%2)*32 + p%32}
        for nb in range(4):
            for m in range(NC_D):
                ps = ps_sc.tile([P, 512], F32, tag="sc")
                for cp in range(2):
                    nc.tensor.matmul(
                        ps[:],
                        wk[:, 2 * cp : 2 * cp + 2, m * P : (m + 1) * P],
                        xt[:, 2 * cp : 2 * cp + 2, nb * 512 : (nb + 1) * 512],
                        start=(cp == 0),
                        stop=(cp == 1),
                        perf_mode=DR,
                    )
                nc.scalar.activation(
                    kt4[:, m, nb * 512 : (nb + 1) * 512],
                    ps[:],
                    AFT.Identity,
                    bias=bkc[:, m : m + 1],
                )

        # ---- V projection -> vext (DoubleRow, bias via broadcast add) ----
        for t in range(NC_S):
            ps = ps_sc.tile([P, 512], F32, tag="sc")
            for cp in range(2):
                nc.tensor.matmul(
                    ps[:],
                    xt[:, 2 * cp : 2 * cp + 2, t * P : (t + 1) * P],
                    wv[:, 2 * cp : 2 * cp + 2, :],
                    start=(cp == 0),
                    stop=(cp == 1),
                    perf_mode=DR,
                )
            nc.vector.tensor_copy(
                vext[:, t, :, 0:DH],
                ps[:].rearrange("p (h d) -> p h d", h=H),
            )

        # ---- Q projection -> qt2 ----
        for m in range(NC_D):
            for nb in range(2):
                ps = ps_sc.tile([P, 512], F32, tag="sc")
                for cp in range(2):
                    nc.tensor.matmul(
                        ps[:],
                        wq[:, 2 * cp : 2 * cp + 2, m * P : (m + 1) * P],
                        xq[:, 2 * cp : 2 * cp + 2, nb * 512 : (nb + 1) * 512],
                        start=(cp == 0),
                        stop=(cp == 1),
                        perf_mode=DR,
                    )
                qslice = slice(nb * 512, (nb + 1) * 512)
                nc.scalar.activation(
                    qt3[0:DH, 2 * m, qslice],
                    ps[0:DH, :],
                    AFT.Identity,
                    bias=bqc[0:DH, m : m + 1],
                )
                nc.vector.tensor_scalar(
                    qt3[DH:P, 2 * m + 1, qslice],
                    ps[DH:P, :],
                    bqc[DH:P, m : m + 1],
                    None,
                    ALU.add,
                )

        # ---- attention ----
        def attention_pair(qb, pair, hooks=()):
            """scores + exp + ctx accumulation for heads (2*pair, 2*pair+1);
            returns the two ctx psum tiles (row 64 = softmax denominator).
            hooks: dict kc -> callback, emitted mid-loop so cross-engine
            work lands in each engine's stream after its deps are met."""
            qs = slice(qb * 512, (qb + 1) * 512)
            cts = [
                ps_ct.tile([P, 512], F32, tag="ct", name=f"ct{qb}{pair}{i}")
                for i in range(2)
            ]
            pt = None
            for kc in range(NC_S):
                if kc in hooks:
                    hooks[kc]()
                j, jj = divmod(kc, 2)
                if jj == 0:
                    pt = ptpool.tile([P, 2, 2, 512], FP8, tag="pt")
                sc = ps_sc.tile([P, 2, 512], F32, tag="sc")
                for hh in range(2):
                    h = 2 * pair + hh
                    nc.tensor.matmul(
                        sc[:, hh, :],
                        kt4[:, pair, kc * P : (kc + 1) * P],
                        qt3[:, h, qs],
                        start=True,
                        stop=True,
                    )
                if kc in EXP_SC:
                    nc.scalar.activation(
                        pt[:, jj, :, :], sc[:], AFT.Exp, scale=SCALE
                    )
                else:
                    nc.vector.tensor_scalar(
                        pt[:, jj, :, :].bitcast(I8),
                        sc[:],
                        SCALE * A8,
                        B8,
                        ALU.mult,
                        ALU.add,
                    )
                if jj == 1 and j < NC_S // 2 - 1:
                    for hh in range(2):
                        nc.tensor.matmul(
                            cts[hh][:, :],
                            vext[:, kc - 1 : kc + 1, 2 * pair + hh, :],
                            pt[:, :, hh, :],
                            start=(j == 0),
                            stop=False,
                            perf_mode=DR,
                        )
            pt_last = pt

            def ctx_last():
                for hh in range(2):
                    nc.tensor.matmul(
                        cts[hh][:, :],
                        vext[:, NC_S - 2 : NC_S, 2 * pair + hh, :],
                        pt_last[:, :, hh, :],
                        start=False,
                        stop=True,
                        perf_mode=DR,
                    )
            return cts, ctx_last

        def evac_pair(qb, pair, cts):
            """Free the ctx psums fast: raw fp8 ctx into ctxt (scaled in
            place later), exact f32 denominator rows, reciprocal via the
            16-partition DMA spread. Returns recrow for the deferred scale."""
            qs = slice(qb * 512, (qb + 1) * 512)
            nc.scalar.activation(ctxt[0:DH, pair, qs], cts[0][0:DH, :], AFT.Copy)
            nc.vector.tensor_copy(ctxt[DH:P, pair, qs], cts[1][0:DH, :])
            denrow = dpool.tile([1, 2, 512], F32, tag="denrow")
            nc.scalar.activation(denrow[:, 0, :], cts[0][DH : DH + 1, :], AFT.Copy)
            nc.vector.tensor_copy(denrow[:, 1, :], cts[1][DH : DH + 1, :])
            dsq = dpool.tile([16, 64], F32, tag="dsq")
            nc.sync.dma_start(dsq[:], denrow[:])
            nc.vector.reciprocal(dsq[:], dsq[:])
            recrow = dpool.tile([2, 512], F32, tag="recrow")
            nc.sync.dma_start(recrow[:], dsq[:])
            return recrow

        def finish_pair(qb, pair, recrow):
            """Deferred ~1 pair: broadcast 1/den and scale ctxt in place."""
            qs = slice(qb * 512, (qb + 1) * 512)
            rb = ps_y.tile([P, 512], F32, tag="y", name=f"rb{qb}{pair}")
            nc.tensor.matmul(
                rb[:],
                sel2[:],
                recrow[:].bitcast(F32R),
                start=True,
                stop=True,
            )
            rb_sb = tpool.tile([P, 512], F32, tag="rb")
            nc.scalar.activation(rb_sb[:], rb[:], AFT.Copy)
            cslice = ctxt[:, pair, qs]
            nc.vector.tensor_tensor(cslice, cslice, rb_sb[:], ALU.mult)

        def outproj_chunk(qc):
            """y[qc*128:(qc+1)*128, :] = ctx @ wo^T + (x + bo); fused stats."""
            ps = ps_y.tile([P, D], F32, tag="y", name=f"y{qc}")
            for cp in range(2):
                nc.tensor.matmul(
                    ps[:],
                    ctxt[:, 2 * cp : 2 * cp + 2, qc * P : (qc + 1) * P],
                    wot[:, 2 * cp : 2 * cp + 2, :],
                    start=(cp == 0),
                    stop=(cp == 1),
                    perf_mode=DR,
                )
            nc.vector.scalar_tensor_tensor(
                y_sb[:, qc, :],
                ps[:],
                0.0,
                xqbo[:, qc, :],
                ALU.bypass,
                ALU.add,
                accum_out=msum[:, qc : qc + 1],
            )
            sq = tpool.tile([P, D], BF16, tag="sq")
            nc.scalar.activation(
                sq[:],
                y_sb[:, qc, :],
                AFT.Square,
                accum_out=msqs[:, qc : qc + 1],
            )

        def ln_stats(qb):
            """rstd/bias for the 4 chunks of query block qb: [128, 4] ops."""
            cs = slice(qb * 4, (qb + 1) * 4)
            mu = dpool.tile([P, 4], F32, tag="mu", name=f"mu{qb}")
            var = dpool.tile([P, 4], F32, tag="var", name=f"var{qb}")
            rstd = singles.tile([P, 4], F32, tag=f"rstd{qb}")
            nmr = singles.tile([P, 4], F32, tag=f"nmr{qb}")
            nc.vector.tensor_scalar_mul(mu[:], msum[:, cs], 1.0 / D)
            nc.vector.tensor_tensor(var[:], mu[:], mu[:], ALU.mult)
            nc.vector.scalar_tensor_tensor(
                var[:], msqs[:, cs], 1.0 / D, var[:], ALU.mult, ALU.subtract
            )
            nc.scalar.activation(rstd[:], var[:], AFT.Sqrt, bias=eps_col[:, 0:1])
            nc.vector.reciprocal(rstd[:], rstd[:])
            nc.vector.scalar_tensor_tensor(
                nmr[:], mu[:], -1.0, rstd[:], ALU.mult, ALU.mult
            )
            return rstd, nmr

        def ln_finalize_chunk(qb, qc, rstd, nmr):
            i = qc - qb * 4
            t1 = tpool.tile([P, D], F32, tag="t1")
            nc.scalar.activation(
                t1[:],
                y_sb[:, qc, :],
                AFT.Identity,
                bias=nmr[:, i : i + 1],
                scale=rstd[:, i : i + 1],
            )
            t2 = tpool.tile([P, D], F32, tag="t2")
            yo = tpool.tile([P, D], F32, tag="yo")
            if qb == 0 or qc % 2 == 0:
                nc.vector.tensor_tensor(t2[:], t1[:], gamma_b[:], ALU.mult)
                eng = nc.gpsimd if qb == 0 else nc.vector
                eng.tensor_tensor(yo[:], t2[:], beta_b[:], ALU.add)
            else:
                nc.gpsimd.tensor_tensor(t2[:], t1[:], gamma_b[:], ALU.mult)
                nc.gpsimd.tensor_tensor(yo[:], t2[:], beta_b[:], ALU.add)
            nc.sync.dma_start(ytd[qc * P : (qc + 1) * P, :], yo[:])

        # attention: pair p's evacuation (kc=4 hook) and normalize/tail work
        # (kc=11 hook) are emitted inside pair p+1/p+2's kc loop, so no
        # engine's in-order stream blocks on a dependency that isn't ready
        cts_d = {}
        rr_d = {}
        ln0 = {}

        def ctxlast_hook(qb, pair):
            def fn():
                cts_d[(qb, pair)][1]()
            return fn

        def evac_hook(qb, pair):
            def fn():
                rr_d[(qb, pair)] = evac_pair(qb, pair, cts_d.pop((qb, pair))[0])
            return fn

        def finish_hook(qb, pair, extra=None):
            def fn():
                finish_pair(qb, pair, rr_d.pop((qb, pair)))
                if extra is not None:
                    extra()
            return fn

        def outproj0_all():
            for qc in range(4):
                outproj_chunk(qc)

        def ln0_block():
            ln0["r"] = ln_stats(0)
            for qc in range(4):
                ln_finalize_chunk(0, qc, *ln0["r"])

        schedule = [
            (0, 0, {}),
            (0, 1, {2: ctxlast_hook(0, 0), 4: evac_hook(0, 0)}),
            (0, 2, {2: ctxlast_hook(0, 1), 4: evac_hook(0, 1),
                    11: finish_hook(0, 0)}),
            (0, 3, {2: ctxlast_hook(0, 2), 4: evac_hook(0, 2),
                    11: finish_hook(0, 1)}),
            (1, 0, {2: ctxlast_hook(0, 3), 4: evac_hook(0, 3),
                    11: finish_hook(0, 2)}),
            (1, 1, {2: ctxlast_hook(1, 0), 4: evac_hook(1, 0),
                    11: finish_hook(0, 3, outproj0_all)}),
            (1, 2, {2: ctxlast_hook(1, 1), 4: evac_hook(1, 1),
                    11: finish_hook(1, 0)}),
            (1, 3, {2: ctxlast_hook(1, 2), 4: evac_hook(1, 2),
                    11: finish_hook(1, 1), 14: ln0_block}),
        ]
        for qb, pair, hooks in schedule:
            cts_d[(qb, pair)] = attention_pair(qb, pair, hooks)
        finish_pair(1, 2, rr_d.pop((1, 2)))
        cts_d[(1, 3)][1]()
        rr_d[(1, 3)] = evac_pair(1, 3, cts_d.pop((1, 3))[0])
        finish_pair(1, 3, rr_d.pop((1, 3)))
        for qc in range(4, 8):
            outproj_chunk(qc)
        r1 = ln_stats(1)
        for qc in range(4, 8):
            ln_finalize_chunk(1, qc, *r1)

    return _patch_serialization(nc)


_nc_cache = None


def _get_nc():
    global _nc_cache
    if _nc_cache is None:
        _nc_cache = build_nc()
    return _nc_cache


def make_in_maps(x, w_q, b_q, w_k, b_k, w_v, b_v, w_o, b_o, ln_gamma, ln_beta):
    import ml_dtypes

    fp8 = ml_dtypes.float8_e4m3
    f8 = lambda a: np.ascontiguousarray(np.asarray(a, np.float32)).astype(fp8)
    f = lambda a: np.ascontiguousarray(np.asarray(a), dtype=np.float32)

    w_q, w_k, w_v, w_o = (np.asarray(w, np.float32) for w in (w_q, w_k, w_v, w_o))
    b_q, b_k = np.asarray(b_q, np.float32), np.asarray(b_k, np.float32)
    shared = dict(
        wkt=f8(w_k.T),
        wqt=f8(w_q.T),
        wvt=f8(w_v.T),
        wot=f8(w_o.T),
        bkc=f(b_k.reshape(NC_D, P)),
        bqc=f(b_q.reshape(NC_D, P)),
        gamma=f(ln_gamma),
        beta=f(ln_beta),
        sel2=np.kron(np.eye(2, dtype=np.float32), np.ones((1, DH), np.float32)),
    )
    x = np.asarray(x, np.float32)
    # b_o + softmax-weight-sum * b_v @ w_o.T both fold into the residual input
    resid_bias = np.asarray(b_o, np.float32) + np.asarray(b_v, np.float32) @ w_o.T
    in_maps = []
    for c in range(NCORES):
        b, half = divmod(c, 2)
        off = half * SQ
        xb = x[b]
        in_maps.append(
            dict(
                xt=f8(xb.T),
                xq=f8(xb[off : off + SQ].T),
                xqbo=np.ascontiguousarray(xb[off : off + SQ] + resid_bias),
                **shared,
            )
        )
    return in_maps


def assemble(results):
    y = np.empty((B, S, D), np.float32)
    for c in range(NCORES):
        b, half = divmod(c, 2)
        off = half * SQ
        y[b, off : off + SQ, :] = results[c]["ytd"]
    return y


def run(inputs, trace=False, **kwargs):
    from concourse.bass_utils import run_bass_kernel_spmd

    nc = _get_nc()
    in_maps = make_in_maps(**inputs)
    res = run_bass_kernel_spmd(
        nc, in_maps, core_ids=list(range(NCORES)), trace=trace, **kwargs
    )
    return assemble(res.results), res


def kernel(**inputs):
    y, _ = run(inputs, trace=False)
    return y


# revision 24
# speedup vs baseline: 1.2481x; 1.0155x over previous
"""MultiHeadAttention + residual + LayerNorm Trainium2 kernel (8 NeuronCores).

Sharding: core c handles batch b = c//2 and query half h = c%2 (1024 queries).
No cross-core communication.

All heavy matmuls run in fp8(e4m3) DoubleRow perf mode (two 128-deep k-tiles
per instruction at 0.5 cycles/row = 4x bf16 throughput per accumulation pair):
  - K^T / Q^T projections with HOST-PERMUTED weight columns so each head's
    64-deep score contraction lands as two 32-row k-tiles on partition group
    (h%4)*32..(h%4+1)*32, free dims (h_hi, dh_hi, token).
  - scores S^T[k, q] = K Q^T as one DoubleRow matmul per (head, key-chunk).
  - softmax exp is split across ScalarE (native Exp -> fp8 out) and DVE
    (Schraudolph: bits = s*A+B as int8, bitcast e4m3; the constant bias only
    multiplies all p's by a constant which cancels in the softmax divide).
  - ctx~^T = [V|1]^T P^T DoubleRow over key-chunk pairs; row 64 = denominator.
  - denominators: DMA-spread to 16 partitions, fast-reciprocal, DMA back,
    PE rank-1 broadcast, fused scale+fp8-cast on DVE -> ctxt.
  - out-projection produces y[q, dout] (queries on partitions) so LayerNorm
    runs over the FREE dim: residual-add with accum_out gives the mean for
    free, ScalarE Square/Pool give sum-of-squares, finalize is
    ScalarE-Identity(scale=rstd, bias=-mu*rstd) + gamma/beta broadcast mults.
  - b_o + x is folded host-side into the residual input; b_q/b_k fold into
    the PSUM->SBUF copies (per-partition bias); b_v is added into V so the
    softmax-weighted average carries it exactly.
"""

import os
from contextlib import ExitStack

import numpy as np

import concourse.bass as bass
import concourse.mybir as mybir
import concourse.tile as tile

B, S, D, H, DH = 4, 2048, 512, 8, 64
SQ = S // 2          # local queries per core
NCORES = 8
P = 128
NC_D = D // P        # 4 chunks of the feature dim
NC_S = S // P        # 16 key chunks
SCALE = float(1.0 / np.sqrt(np.float32(D)))
EPS = 1e-5
LN2 = float(np.log(2.0))
A8 = 8.0 / LN2                     # Schraudolph slope for e4m3 bit space
B8 = 7.0 * 8.0 - 0.34 + 0.5        # bias: exp-field offset, ripple center,
                                   # +0.5 for truncating converts (any
                                   # constant shift cancels in softmax)

F32 = mybir.dt.float32
F32R = mybir.dt.float32r
BF16 = mybir.dt.bfloat16
FP8 = mybir.dt.float8e4
I8 = mybir.dt.int8
ALU = mybir.AluOpType
AFT = mybir.ActivationFunctionType
DR = mybir.MatmulPerfMode.DoubleRow

# exp engine rotation per key-chunk: ScalarE native exp is ~1.3x faster per
# chunk than the DVE Schraudolph, so it takes 10 of 16
EXP_SC = (0, 2, 4, 5, 7, 9, 11, 12, 14)
# remaining 7 on DVE: (1, 3, 6, 8, 10, 13, 15) - the final two kc land on
# different engines so the last pt completes without serializing


def _split_multiwait_json(bir, cap=1):
    """The walrus build here encodes at most one sync-wait command per
    instruction. Hoist excess waits onto preceding single-wait NoOps on the
    same engine - engine streams execute in order, so waiting earlier is
    always safe."""
    n = 0
    for fn in bir.get("functions", []):
        for bb in fn.get("blocks", []):
            out = []
            for ins in bb.get("instructions", []):
                si = ins.get("sync_info")
                waits = (si or {}).get("on_wait") or []
                if len(waits) > cap:
                    extra, si["on_wait"] = waits[:-cap], waits[-cap:]
                    for i in range(0, len(extra), cap):
                        n += 1
                        out.append(
                            {
                                "debug": ins.get("debug", 0),
                                "engine": ins["engine"],
                                "ins": [],
                                "outs": [],
                                "name": f"{ins['name']}-wsplit{n}",
                                "opcode": "NoOp",
                                "sync_info": {
                                    "on_wait": extra[i : i + cap],
                                    "on_update": [],
                                },
                            }
                        )
                out.append(ins)
            bb["instructions"] = out
    return bir


def _patch_serialization(nc):
    import orjson

    orig = nc.to_json_bytes

    def to_json_bytes_split():
        return orjson.dumps(_split_multiwait_json(orjson.loads(orig())))

    nc.to_json_bytes = to_json_bytes_split
    return nc


def build_nc():
    nc = bass.Bass("TRN2", target_bir_lowering=False)

    xt_d = nc.dram_tensor("xt", [D, S], FP8, kind="ExternalInput")
    xq_d = nc.dram_tensor("xq", [D, SQ], FP8, kind="ExternalInput")
    xqbo_d = nc.dram_tensor("xqbo", [SQ, D], BF16, kind="ExternalInput")
    wk_d = nc.dram_tensor("wkt", [D, D], FP8, kind="ExternalInput")
    wq_d = nc.dram_tensor("wqt", [D, D], FP8, kind="ExternalInput")
    wv_d = nc.dram_tensor("wvt", [D, D], FP8, kind="ExternalInput")
    wo_d = nc.dram_tensor("wot", [D, D], FP8, kind="ExternalInput")
    bkc_d = nc.dram_tensor("bkc", [NC_D, P], F32, kind="ExternalInput")
    bqc_d = nc.dram_tensor("bqc", [NC_D, P], F32, kind="ExternalInput")
    sel2_d = nc.dram_tensor("sel2", [2, P], F32R, kind="ExternalInput")
    gamma_d = nc.dram_tensor("gamma", [D], F32R, kind="ExternalInput")
    beta_d = nc.dram_tensor("beta", [D], F32R, kind="ExternalInput")
    ytd = nc.dram_tensor("ytd", [SQ, D], F32, kind="ExternalOutput")

    with (
        tile.TileContext(nc) as tc,
        ExitStack() as ctx,
        nc.allow_low_precision(reason="fp8 matmuls feed a diffuse softmax"),
    ):
        singles = ctx.enter_context(tc.tile_pool(name="singles", bufs=1))
        ptpool = ctx.enter_context(tc.tile_pool(name="ptpool", bufs=3))
        dpool = ctx.enter_context(tc.tile_pool(name="dpool", bufs=2))
        tpool = ctx.enter_context(tc.tile_pool(name="tpool", bufs=2))
        ps_sc = ctx.enter_context(tc.tile_pool(name="ps_sc", bufs=2, space="PSUM"))
        ps_ct = ctx.enter_context(tc.tile_pool(name="ps_ct", bufs=3, space="PSUM"))
        ps_y = ctx.enter_context(tc.tile_pool(name="ps_y", bufs=1, space="PSUM"))

        # ---- input DMAs (K-proj path first) ----
        xt = singles.tile([P, NC_D, S], FP8)
        wk = singles.tile([P, NC_D, D], FP8)
        bkc = singles.tile([P, NC_D], F32)
        bqc = singles.tile([P, NC_D], F32)
        nc.sync.dma_start(wk[:], wk_d[:, :].rearrange("(c p) f -> p c f", p=P))
        nc.sync.dma_start(bkc[:], bkc_d[:, :].rearrange("m p -> p m"))
        nc.sync.dma_start(bqc[:], bqc_d[:, :].rearrange("m p -> p m"))
        for i in range(4):
            ts_ = slice(i * 512, (i + 1) * 512)
            nc.sync.dma_start(
                xt[:, :, ts_],
                xt_d[:, :].rearrange("(c p) t -> p c t", p=P)[:, :, ts_],
            )
        wv = singles.tile([P, NC_D, D], FP8)
        nc.sync.dma_start(wv[:], wv_d[:, :].rearrange("(c p) f -> p c f", p=P))
        wq = singles.tile([P, NC_D, D], FP8)
        nc.sync.dma_start(wq[:], wq_d[:, :].rearrange("(c p) f -> p c f", p=P))
        xq = singles.tile([P, NC_D, SQ], FP8)
        nc.sync.dma_start(xq[:], xq_d[:, :].rearrange("(c p) t -> p c t", p=P))
        wot = singles.tile([P, NC_D, D], FP8)
        nc.sync.dma_start(wot[:], wo_d[:, :].rearrange("(c p) f -> p c f", p=P))
        xqbo = singles.tile([P, SQ // P, D], BF16)
        nc.sync.dma_start(
            xqbo[:], xqbo_d[:, :].rearrange("(qc p) d -> p qc d", p=P)
        )
        gamma_row = singles.tile([1, D], F32R)
        beta_row = singles.tile([1, D], F32R)
        nc.sync.dma_start(gamma_row[:], gamma_d[:][None, :])
        nc.sync.dma_start(beta_row[:], beta_d[:][None, :])

        # ---- persistent SBUF ----
        kt4 = singles.tile([P, NC_D, S], FP8)   # [dk%128, dk//128, t]
        qt3 = singles.tile([P, H, SQ], FP8)     # head h real rows at (h%2)*64
        vext = singles.tile([P, NC_S, H, P], FP8)
        ctxt = singles.tile([P, NC_D, SQ], FP8)    # [dv%128, h//2, q]
        y_sb = singles.tile([P, SQ // P, D], F32)  # residual+proj, LN input
        msum = singles.tile([P, SQ // P], F32)
        msqs = singles.tile([P, SQ // P], F32)

        # consts
        ones_col_r = singles.tile([1, P], F32R)
        eps_col = singles.tile([P, 1], F32)
        f32tmp = singles.tile([P, 512], F32)
        nc.vector.memset(f32tmp[:], 1.0)
        nc.vector.tensor_copy(ones_col_r[:], f32tmp[0:1, 0:P])
        nc.vector.memset(eps_col[:], EPS)
        nc.vector.memset(vext[:, :, :, DH:P], 1.0)  # ones -> denominators
        nc.vector.memset(qt3[0:DH, 1::2, :], 0.0)  # zero rows select the
        nc.vector.memset(qt3[DH:P, 0::2, :], 0.0)  # head in score matmuls
        sel2 = singles.tile([2, P], F32R)  # rank-2 lhsT: head parity -> rows
        nc.sync.dma_start(sel2[:], sel2_d[:, :])

        # broadcast rows -> [128, 512] via rank-1 matmuls
        gamma_b = singles.tile([P, D], F32)
        beta_b = singles.tile([P, D], F32)
        for row, dst in ((gamma_row, gamma_b), (beta_row, beta_b)):
            ps = ps_y.tile([P, D], F32, tag="y")
            nc.tensor.matmul(ps[:], ones_col_r[:], row[:], start=True, stop=True)
            nc.scalar.activation(dst[:], ps[:], AFT.Copy)

        # ---- K projection -> kt2 (DoubleRow, bias folded into copy) ----
        # m-chunk m covers dk rows {(m//2*4 + p//32)*64 + (m%2)*32 + p%32}
        for nb in range(4):
            for m in range(NC_D):
                ps = ps_sc.tile([P, 512], F32, tag="sc")
                for cp in range(2):
                    nc.tensor.matmul(
                        ps[:],
                        wk[:, 2 * cp : 2 * cp + 2, m * P : (m + 1) * P],
                        xt[:, 2 * cp : 2 * cp + 2, nb * 512 : (nb + 1) * 512],
                        start=(cp == 0),
                        stop=(cp == 1),
                        perf_mode=DR,
                    )
                nc.scalar.activation(
                    kt4[:, m, nb * 512 : (nb + 1) * 512],
                    ps[:],
                    AFT.Identity,
                    bias=bkc[:, m : m + 1],
                )

        # ---- V projection -> vext (DoubleRow, bias via broadcast add) ----
        for t in range(NC_S):
            ps = ps_sc.tile([P, 512], F32, tag="sc")
            for cp in range(2):
                nc.tensor.matmul(
                    ps[:],
                    xt[:, 2 * cp : 2 * cp + 2, t * P : (t + 1) * P],
                    wv[:, 2 * cp : 2 * cp + 2, :],
                    start=(cp == 0),
                    stop=(cp == 1),
                    perf_mode=DR,
                )
            nc.vector.tensor_copy(
                vext[:, t, :, 0:DH],
                ps[:].rearrange("p (h d) -> p h d", h=H),
            )

        # ---- Q projection -> qt2 ----
        for m in range(NC_D):
            for nb in range(2):
                ps = ps_sc.tile([P, 512], F32, tag="sc")
                for cp in range(2):
                    nc.tensor.matmul(
                        ps[:],
                        wq[:, 2 * cp : 2 * cp + 2, m * P : (m + 1) * P],
                        xq[:, 2 * cp : 2 * cp + 2, nb * 512 : (nb + 1) * 512],
                        start=(cp == 0),
                        stop=(cp == 1),
                        perf_mode=DR,
                    )
                qslice = slice(nb * 512, (nb + 1) * 512)
                nc.scalar.activation(
                    qt3[0:DH, 2 * m, qslice],
                    ps[0:DH, :],
                    AFT.Identity,
                    bias=bqc[0:DH, m : m + 1],
                )
                nc.vector.tensor_scalar(
                    qt3[DH:P, 2 * m + 1, qslice],
                    ps[DH:P, :],
                    bqc[DH:P, m : m + 1],
                    None,
                    ALU.add,
                )

        # ---- attention ----
        def attention_pair(qb, pair, hooks=()):
            """scores + exp + ctx accumulation for heads (2*pair, 2*pair+1);
            returns the two ctx psum tiles (row 64 = softmax denominator).
            hooks: dict kc -> callback, emitted mid-loop so cross-engine
            work lands in each engine's stream after its deps are met."""
            qs = slice(qb * 512, (qb + 1) * 512)
            cts = [
                ps_ct.tile([P, 512], F32, tag="ct", name=f"ct{qb}{pair}{i}")
                for i in range(2)
            ]
            pt = None
            for kc in range(NC_S):
                if kc in hooks:
                    hooks[kc]()
                j, jj = divmod(kc, 2)
                if jj == 0:
                    pt = ptpool.tile([P, 2, 2, 512], FP8, tag="pt")
                sc = ps_sc.tile([P, 2, 512], F32, tag="sc")
                for hh in range(2):
                    h = 2 * pair + hh
                    nc.tensor.matmul(
                        sc[:, hh, :],
                        kt4[:, pair, kc * P : (kc + 1) * P],
                        qt3[:, h, qs],
                        start=True,
                        stop=True,
                    )
                if kc in EXP_SC:
                    nc.scalar.activation(
                        pt[:, jj, :, :], sc[:], AFT.Exp, scale=SCALE
                    )
                else:
                    nc.vector.tensor_scalar(
                        pt[:, jj, :, :].bitcast(I8),
                        sc[:],
                        SCALE * A8,
                        B8,
                        ALU.mult,
                        ALU.add,
                    )
                if jj == 1 and j < NC_S // 2 - 1:
                    for hh in range(2):
                        nc.tensor.matmul(
                            cts[hh][:, :],
                            vext[:, kc - 1 : kc + 1, 2 * pair + hh, :],
                            pt[:, :, hh, :],
                            start=(j == 0),
                            stop=False,
                            perf_mode=DR,
                        )
            pt_last = pt

            def ctx_last():
                for hh in range(2):
                    nc.tensor.matmul(
                        cts[hh][:, :],
                        vext[:, NC_S - 2 : NC_S, 2 * pair + hh, :],
                        pt_last[:, :, hh, :],
                        start=False,
                        stop=True,
                        perf_mode=DR,
                    )
            return cts, ctx_last

        def evac_pair(qb, pair, cts):
            """Free the ctx psums fast: raw fp8 ctx into ctxt (scaled in
            place later), exact f32 denominator rows, reciprocal via the
            16-partition DMA spread. Returns recrow for the deferred scale."""
            qs = slice(qb * 512, (qb + 1) * 512)
            nc.scalar.activation(ctxt[0:DH, pair, qs], cts[0][0:DH, :], AFT.Copy)
            nc.vector.tensor_copy(ctxt[DH:P, pair, qs], cts[1][0:DH, :])
            denrow = dpool.tile([1, 2, 512], F32, tag="denrow")
            nc.scalar.activation(denrow[:, 0, :], cts[0][DH : DH + 1, :], AFT.Copy)
            nc.vector.tensor_copy(denrow[:, 1, :], cts[1][DH : DH + 1, :])
            dsq = dpool.tile([16, 64], F32, tag="dsq")
            nc.sync.dma_start(dsq[:], denrow[:])
            nc.vector.reciprocal(dsq[:], dsq[:])
            recrow = dpool.tile([2, 512], F32, tag="recrow")
            nc.sync.dma_start(recrow[:], dsq[:])
            return recrow

        def finish_pair(qb, pair, recrow):
            """Deferred ~1 pair: broadcast 1/den and scale ctxt in place."""
            qs = slice(qb * 512, (qb + 1) * 512)
            rb = ps_y.tile([P, 512], F32, tag="y", name=f"rb{qb}{pair}")
            nc.tensor.matmul(
                rb[:],
                sel2[:],
                recrow[:].bitcast(F32R),
                start=True,
                stop=True,
            )
            rb_sb = tpool.tile([P, 512], F32, tag="rb")
            nc.scalar.activation(rb_sb[:], rb[:], AFT.Copy)
            cslice = ctxt[:, pair, qs]
            nc.vector.tensor_tensor(cslice, cslice, rb_sb[:], ALU.mult)

        def outproj_chunk(qc):
            """y[qc*128:(qc+1)*128, :] = ctx @ wo^T + (x + bo); fused stats."""
            ps = ps_y.tile([P, D], F32, tag="y", name=f"y{qc}")
            for cp in range(2):
                nc.tensor.matmul(
                    ps[:],
                    ctxt[:, 2 * cp : 2 * cp + 2, qc * P : (qc + 1) * P],
                    wot[:, 2 * cp : 2 * cp + 2, :],
                    start=(cp == 0),
                    stop=(cp == 1),
                    perf_mode=DR,
                )
            nc.vector.scalar_tensor_tensor(
                y_sb[:, qc, :],
                ps[:],
                0.0,
                xqbo[:, qc, :],
                ALU.bypass,
                ALU.add,
                accum_out=msum[:, qc : qc + 1],
            )
            sq = tpool.tile([P, D], BF16, tag="sq")
            if qc < 4:
                nc.vector.scalar_tensor_tensor(
                    sq[:],
                    y_sb[:, qc, :],
                    0.0,
                    y_sb[:, qc, :],
                    ALU.bypass,
                    ALU.mult,
                    accum_out=msqs[:, qc : qc + 1],
                )
            else:
                nc.scalar.activation(
                    sq[:],
                    y_sb[:, qc, :],
                    AFT.Square,
                    accum_out=msqs[:, qc : qc + 1],
                )

        def ln_stats(qb):
            """rstd/bias for the 4 chunks of query block qb: [128, 4] ops."""
            cs = slice(qb * 4, (qb + 1) * 4)
            mu = dpool.tile([P, 4], F32, tag="mu", name=f"mu{qb}")
            var = dpool.tile([P, 4], F32, tag="var", name=f"var{qb}")
            rstd = singles.tile([P, 4], F32, tag=f"rstd{qb}")
            nmr = singles.tile([P, 4], F32, tag=f"nmr{qb}")
            nc.vector.tensor_scalar_mul(mu[:], msum[:, cs], 1.0 / D)
            nc.vector.tensor_tensor(var[:], mu[:], mu[:], ALU.mult)
            nc.vector.scalar_tensor_tensor(
                var[:], msqs[:, cs], 1.0 / D, var[:], ALU.mult, ALU.subtract
            )
            nc.scalar.activation(rstd[:], var[:], AFT.Sqrt, bias=eps_col[:, 0:1])
            nc.vector.reciprocal(rstd[:], rstd[:])
            nc.vector.scalar_tensor_tensor(
                nmr[:], mu[:], -1.0, rstd[:], ALU.mult, ALU.mult
            )
            return rstd, nmr

        def ln_finalize_chunk(qb, qc, rstd, nmr):
            i = qc - qb * 4
            t1 = tpool.tile([P, D], F32, tag="t1")
            nc.scalar.activation(
                t1[:],
                y_sb[:, qc, :],
                AFT.Identity,
                bias=nmr[:, i : i + 1],
                scale=rstd[:, i : i + 1],
            )
            t2 = tpool.tile([P, D], F32, tag="t2")
            yo = tpool.tile([P, D], F32, tag="yo")
            if qb == 0:
                nc.gpsimd.tensor_tensor(t2[:], t1[:], gamma_b[:], ALU.mult)
                nc.gpsimd.tensor_tensor(yo[:], t2[:], beta_b[:], ALU.add)
            else:
                nc.vector.tensor_tensor(t2[:], t1[:], gamma_b[:], ALU.mult)
                nc.vector.tensor_tensor(yo[:], t2[:], beta_b[:], ALU.add)
            nc.sync.dma_start(ytd[qc * P : (qc + 1) * P, :], yo[:])

        # attention: pair p's evacuation (kc=4 hook) and normalize/tail work
        # (kc=11 hook) are emitted inside pair p+1/p+2's kc loop, so no
        # engine's in-order stream blocks on a dependency that isn't ready
        cts_d = {}
        rr_d = {}
        ln0 = {}

        def ctxlast_hook(qb, pair):
            def fn():
                cts_d[(qb, pair)][1]()
            return fn

        def evac_hook(qb, pair):
            def fn():
                rr_d[(qb, pair)] = evac_pair(qb, pair, cts_d.pop((qb, pair))[0])
            return fn

        def finish_hook(qb, pair, extra=None):
            def fn():
                finish_pair(qb, pair, rr_d.pop((qb, pair)))
                if extra is not None:
                    extra()
            return fn

        def outproj0_all():
            for qc in range(4):
                outproj_chunk(qc)

        def ln0_block():
            ln0["r"] = ln_stats(0)
            for qc in range(4):
                ln_finalize_chunk(0, qc, *ln0["r"])

        schedule = [
            (0, 0, {}),
            (0, 1, {2: ctxlast_hook(0, 0), 4: evac_hook(0, 0)}),
            (0, 2, {2: ctxlast_hook(0, 1), 4: evac_hook(0, 1),
                    11: finish_hook(0, 0)}),
            (0, 3, {2: ctxlast_hook(0, 2), 4: evac_hook(0, 2),
                    11: finish_hook(0, 1)}),
            (1, 0, {2: ctxlast_hook(0, 3), 4: evac_hook(0, 3),
                    11: finish_hook(0, 2)}),
            (1, 1, {2: ctxlast_hook(1, 0), 4: evac_hook(1, 0),
                    11: finish_hook(0, 3, outproj0_all)}),
            (1, 2, {2: ctxlast_hook(1, 1), 4: evac_hook(1, 1),
                    11: finish_hook(1, 0)}),
            (1, 3, {2: ctxlast_hook(1, 2), 4: evac_hook(1, 2),
                    11: finish_hook(1, 1), 14: ln0_block}),
        ]
        for qb, pair, hooks in schedule:
            cts_d[(qb, pair)] = attention_pair(qb, pair, hooks)
        finish_pair(1, 2, rr_d.pop((1, 2)))
        cts_d[(1, 3)][1]()
        rr_d[(1, 3)] = evac_pair(1, 3, cts_d.pop((1, 3))[0])
        finish_pair(1, 3, rr_d.pop((1, 3)))
        for qc in range(4, 8):
            outproj_chunk(qc)
        r1 = ln_stats(1)
        for qc in range(4, 8):
            ln_finalize_chunk(1, qc, *r1)

    return _patch_serialization(nc)


_nc_cache = None


def _get_nc():
    global _nc_cache
    if _nc_cache is None:
        _nc_cache = build_nc()
    return _nc_cache


def make_in_maps(x, w_q, b_q, w_k, b_k, w_v, b_v, w_o, b_o, ln_gamma, ln_beta):
    import ml_dtypes

    fp8 = ml_dtypes.float8_e4m3
    f8 = lambda a: np.ascontiguousarray(np.asarray(a, np.float32)).astype(fp8)
    f = lambda a: np.ascontiguousarray(np.asarray(a), dtype=np.float32)

    w_q, w_k, w_v, w_o = (np.asarray(w, np.float32) for w in (w_q, w_k, w_v, w_o))
    b_q, b_k = np.asarray(b_q, np.float32), np.asarray(b_k, np.float32)
    shared = dict(
        wkt=f8(w_k.T),
        wqt=f8(w_q.T),
        wvt=f8(w_v.T),
        wot=f8(w_o.T),
        bkc=f(b_k.reshape(NC_D, P)),
        bqc=f(b_q.reshape(NC_D, P)),
        gamma=f(ln_gamma),
        beta=f(ln_beta),
        sel2=np.kron(np.eye(2, dtype=np.float32), np.ones((1, DH), np.float32)),
    )
    x = np.asarray(x, np.float32)
    # b_o + softmax-weight-sum * b_v @ w_o.T both fold into the residual input
    resid_bias = np.asarray(b_o, np.float32) + np.asarray(b_v, np.float32) @ w_o.T
    in_maps = []
    for c in range(NCORES):
        b, half = divmod(c, 2)
        off = half * SQ
        xb = x[b]
        in_maps.append(
            dict(
                xt=f8(xb.T),
                xq=f8(xb[off : off + SQ].T),
                xqbo=np.ascontiguousarray(xb[off : off + SQ] + resid_bias).astype(ml_dtypes.bfloat16),
                **shared,
            )
        )
    return in_maps


def assemble(results):
    y = np.empty((B, S, D), np.float32)
    for c in range(NCORES):
        b, half = divmod(c, 2)
        off = half * SQ
        y[b, off : off + SQ, :] = results[c]["ytd"]
    return y


def run(inputs, trace=False, **kwargs):
    from concourse.bass_utils import run_bass_kernel_spmd

    nc = _get_nc()
    in_maps = make_in_maps(**inputs)
    res = run_bass_kernel_spmd(
        nc, in_maps, core_ids=list(range(NCORES)), trace=trace, **kwargs
    )
    return assemble(res.results), res


def kernel(**inputs):
    y, _ = run(inputs, trace=False)
    return y
